# revision 1
# baseline (speedup 1.0000x reference)
"""Trainium2 Bass kernel for nn_Attention_43516608643501.

Cross-attention: Q = out_d [T,B,H]; K = V = sum of fwd/bwd halves of out_e
-> [S,B,H]; scores = Q @ K^T per batch (contraction over H, no scaling);
softmax over the source dim S; context = P @ V -> output [T,B,H].

Sharding: data-parallel over batch (dim 1): 2 batches per core x 8 cores,
no cross-core communication.

Design: one flattened software pipeline over 32 (batch, t-tile)
iterations.  Scores are computed in [t_partition, s_free] tiles so the
softmax max/sum are free-dim reductions (DVE reduce_max + ACT exp
accum_out).  P is transposed back to [s,t] on the PE (fp16, 1 cyc/row).
Per iteration g the PE queue is:
  [P-transposes(g-1)] [input-prep transposes] [MM1(g)] [MM2(g-1)]
so the PSUM->SBUF copies of P^T(g-1) (DVE) hide under MM1(g), and the
input-prep transposes' DMA+add dependencies were issued >=2 iterations
earlier.  exp(g) is queued on ACT before out-scale(g-1) so the strict
ACT FIFO can't delay MM1(g+1)'s PSUM-bank reuse.  Batch 1's input prep
is software-pipelined into batch 0's tail (oe DMAs at tiles 3..10, the
fwd+bwd adds -- on the otherwise idle GpSimd engine -- at 5..12, PE
transposes at 7..14), so the batch boundary has no pipeline bubble.
The od f32->f16 casts also run on GpSimd, keeping the DVE FIFO clear
for the P^T copies that gate MM2.

Numerics: both matmuls run in fp16 (full PE rate; fp16's 4.9e-4 rounding
vs bf16's 4e-3 matters because the scores carry no 1/sqrt(H) scaling, so
near-ties in the softmax amplify score error by exp()).
"""

import numpy as np
from contextlib import ExitStack

S, T, B, H = 2048, 2048, 16, 512
NCORES = 8
BLOC = B // NCORES  # batches per core
P128 = 128
NS = S // P128  # 16 s-tiles
NT = T // P128  # 16 t-tiles
NH = H // P128  # 4 h-chunks of the contraction
SC = 512  # s-chunk width (scores tile columns)
NSC = S // SC  # 4 s-chunks per t-tile

_cached_nc = None


def _build():
    import concourse.bacc as bacc
    import concourse.tile as tile
    from concourse import mybir
    from concourse.masks import make_identity

    f32 = mybir.dt.float32
    f16 = mybir.dt.float16

    nc = bacc.Bacc(None, target_bir_lowering=False)
    d_oe = nc.dram_tensor("out_e", [S, BLOC, 2 * H], f32, kind="ExternalInput")
    d_od = nc.dram_tensor("out_d", [T, BLOC, H], f32, kind="ExternalInput")
    d_out = nc.dram_tensor("out", [T, BLOC, H], f32, kind="ExternalOutput")

    with ExitStack() as ctx:
        tc = ctx.enter_context(tile.TileContext(nc))
        singles = ctx.enter_context(tc.tile_pool(name="singles", bufs=1))
        loads = ctx.enter_context(tc.tile_pool(name="loads", bufs=8))
        persist = ctx.enter_context(tc.tile_pool(name="persist", bufs=2))
        work = ctx.enter_context(tc.tile_pool(name="work", bufs=4))
        ptile = ctx.enter_context(tc.tile_pool(name="ptile", bufs=2))
        outs = ctx.enter_context(tc.tile_pool(name="outs", bufs=3))
        small = ctx.enter_context(tc.tile_pool(name="small", bufs=3))
        # PSUM: 8 banks = ps_s0..3 (4) + tr (2) + ps_c (2)
        ps_s_pool = ctx.enter_context(tc.tile_pool(name="ps_s_pool", bufs=1, space="PSUM"))
        ps_tr = ctx.enter_context(tc.tile_pool(name="ps_tr", bufs=2, space="PSUM"))
        ps_cp = ctx.enter_context(tc.tile_pool(name="ps_cp", bufs=2, space="PSUM"))

        id16 = singles.tile([P128, P128], f16)
        make_identity(nc, id16)

        # per-batch persistent tile handles (persist pool tags rotate
        # bufs=2 slots, so consecutive batches double-buffer)
        st = [dict(oeT=[None] * NSC, odT=[None] * NT, nat=[None] * NS,
                   odf=[None] * NT) for _ in range(BLOC)]

        raws = {}

        def oe_dma(b, k, eng=None):
            raw = loads.tile([P128, 2 * H], f32, tag="raw", name="raw", bufs=10)
            (eng or nc.sync).dma_start(
                out=raw, in_=d_oe[k * P128:(k + 1) * P128, b, :]
            )
            raws[(b, k)] = raw

        def oe_add(b, k):
            raw = raws.pop((b, k))
            nat = persist.tile([P128, H], f16, tag=f"oenat{k}", name=f"oenat{k}")
            nc.gpsimd.tensor_add(nat, raw[:, 0:H], raw[:, H:2 * H])
            st[b]["nat"][k] = nat

        def oe_load(b, k):
            if (b, k) not in raws:
                oe_dma(b, k)
            oe_add(b, k)

        def oe_tr(b, k):
            nat = st[b]["nat"][k]
            ci, j = k // 4, k % 4
            if st[b]["oeT"][ci] is None or j == 0:
                st[b]["oeT"][ci] = persist.tile(
                    [P128, NH, SC], f16, tag=f"oeT{ci}", name=f"oeT{ci}"
                )
            trp = ps_tr.tile([P128, H], f16, tag="tr", name="tr_oe")
            for hc in range(NH):
                nc.tensor.transpose(
                    trp[:, hc * P128:(hc + 1) * P128],
                    nat[:, hc * P128:(hc + 1) * P128],
                    id16,
                )
            dst = st[b]["oeT"][ci][:, :, j * P128:(j + 1) * P128]
            src = trp.rearrange("p (h s) -> p h s", h=NH)
            nc.scalar.copy(dst, src)

        def od_load(b, tt):
            odr = loads.tile([P128, H], f32, tag="odr", name="odr")
            nc.sync.dma_start(out=odr, in_=d_od[tt * P128:(tt + 1) * P128, b, :])
            odf = work.tile([P128, H], f16, tag="odf", name="odf")
            nc.gpsimd.tensor_copy(odf, odr)
            st[b]["odf"][tt] = odf

        def od_tr(b, tt):
            odf = st[b]["odf"][tt]
            trp = ps_tr.tile([P128, H], f16, tag="tr", name="tr_od")
            for hc in range(NH):
                nc.tensor.transpose(
                    trp[:, hc * P128:(hc + 1) * P128],
                    odf[:, hc * P128:(hc + 1) * P128],
                    id16,
                )
            odT = persist.tile([P128, NH, P128], f16, tag=f"odT{tt}", name=f"odT{tt}")
            nc.scalar.copy(odT, trp.rearrange("p (h t) -> p h t", h=NH))
            st[b]["odT"][tt] = odT

        def ptr_stage(prev):
            """PE transposes of P(g-1) [t,s]->[s,t] + DVE PSUM->SBUF copies."""
            _, _, pts, _ = prev
            pTs = []
            for half in range(2):
                ptr = ps_tr.tile([P128, 2, SC], f16, tag="tr", name="ptr")
                for sub in range(2):
                    ci = 2 * half + sub
                    for j in range(SC // P128):
                        nc.tensor.transpose(
                            ptr[:, sub, j * P128:(j + 1) * P128],
                            pts[ci][:, j * P128:(j + 1) * P128],
                            id16,
                        )
                pT = ptile.tile([P128, 2 * SC], f16, tag=f"pT{half}", name=f"pT{half}")
                nc.vector.tensor_copy(pT, ptr.rearrange("p a b -> p (a b)"))
                pTs.append(pT)
            return pTs

        def mm1_chunk(b, tt, ci, mx, pss_list):
            pss = ps_s_pool.tile([P128, SC], f32, tag=f"ps_s{ci}", name=f"ps_s{ci}")
            odT = st[b]["odT"][tt]
            oeT = st[b]["oeT"][ci]
            for hc in range(NH):
                nc.tensor.matmul(
                    pss,
                    odT[:, hc, :],
                    oeT[:, hc, :],
                    start=(hc == 0),
                    stop=(hc == NH - 1),
                )
            nc.vector.reduce_max(mx[:, ci:ci + 1], pss, axis=mybir.AxisListType.X)
            pss_list.append(pss)

        def mm1(b, tt):
            mx = small.tile([P128, NSC], f32, tag="mx", name="mx")
            pss_list = []
            for ci in range(NSC):
                mm1_chunk(b, tt, ci, mx, pss_list)
            return mx, pss_list

        def softmax_exp(b, tt, mx, pss_list, order=None):
            m = small.tile([P128, 1], f32, tag="m", name="m")
            nc.vector.reduce_max(m, mx, axis=mybir.AxisListType.X)
            neg_m = small.tile([P128, 1], f32, tag="neg_m", name="neg_m")
            nc.vector.tensor_scalar_mul(neg_m, m, -1.0)
            lacc = small.tile([P128, NSC], f32, tag="lacc", name="lacc")
            pts = [None] * NSC
            for ci in (order if order is not None else range(NSC)):
                pt = ptile.tile([P128, SC], f16, tag=f"pt{ci}", name=f"pt{ci}")
                nc.scalar.activation(
                    pt, pss_list[ci],
                    mybir.ActivationFunctionType.Exp,
                    bias=neg_m, scale=1.0,
                    accum_out=lacc[:, ci:ci + 1],
                )
                pts[ci] = pt
            l = small.tile([P128, 1], f32, tag="l", name="l")
            nc.vector.reduce_sum(l, lacc, axis=mybir.AxisListType.X)
            linv = small.tile([P128, 1], f32, tag="linv", name="linv")
            nc.vector.reciprocal(linv, l)
            return b, tt, pts, linv

        def mm2_mms(prev, pTs):
            pb, _, _, _ = prev
            ps_c = ps_cp.tile([P128, H], f32, tag="ps_c", name="ps_c")
            nat = st[pb]["nat"]
            for k in range(NS):
                nc.tensor.matmul(
                    ps_c,
                    pTs[k // 8][:, (k % 8) * P128:(k % 8 + 1) * P128],
                    nat[k],
                    start=(k == 0), stop=(k == NS - 1),
                )
            return ps_c

        def mm2_out(prev, ps_c):
            pb, ptt, _, plinv = prev
            ot = outs.tile([P128, H], f32, tag="ot", name="ot")
            nc.scalar.activation(
                ot, ps_c, mybir.ActivationFunctionType.Identity,
                bias=0.0, scale=plinv,
            )
            nc.sync.dma_start(
                out=d_out[ptt * P128:(ptt + 1) * P128, pb, :], in_=ot
            )

        def mm2(prev, pTs):
            mm2_out(prev, mm2_mms(prev, pTs))

        # ---- intro: batch 0 tile 0, chunk-interleaved with the oe loads
        # (DMA-bound; the PE stalls here are unavoidable) ----
        # first two oe DMAs go out on the scalar hwdge queue so their
        # descriptor generation doesn't serialize behind od0 on sync
        oe_dma(0, 0, eng=nc.scalar)
        oe_dma(0, 1, eng=nc.scalar)
        od_load(0, 0)
        mx0 = small.tile([P128, NSC], f32, tag="mx", name="mx")
        mx1 = small.tile([P128, NSC], f32, tag="mx", name="mx1")
        pss0 = []
        ps1_hi = {}
        HC2 = SC // 2
        for ci in range(NSC):
            if ci < 2:
                # chunks 0/1 land in the fully DMA-starved region: run
                # N=256 sub-matmuls per 2-tile arrival so the PE starts
                # (and HAM warms) as early as possible
                pss = ps_s_pool.tile(
                    [P128, SC], f32, tag=f"ps_s{ci}", name=f"ps_s{ci}"
                )
                for j in range(2):
                    for k in range(4 * ci + 2 * j, 4 * ci + 2 * j + 2):
                        oe_load(0, k)
                        oe_tr(0, k)
                    if ci == 0 and j == 0:
                        od_tr(0, 0)
                        od_load(0, 1)
                    for hc in range(NH):
                        nc.tensor.matmul(
                            pss[:, j * HC2:(j + 1) * HC2],
                            st[0]["odT"][0][:, hc, :],
                            st[0]["oeT"][ci][:, hc, j * HC2:(j + 1) * HC2],
                            start=(hc == 0),
                            stop=(hc == NH - 1),
                            skip_group_check=True,
                        )
                nc.vector.reduce_max(
                    mx0[:, ci:ci + 1], pss, axis=mybir.AxisListType.X
                )
                pss0.append(pss)
                if ci == 1:
                    od_tr(0, 1)
                continue
            for k in range(4 * ci, 4 * ci + 4):
                oe_load(0, k)
                oe_tr(0, k)
            mm1_chunk(0, 0, ci, mx0, pss0)
            if ci >= 2:
                # tile 1's chunks 2/3 fill the DMA-paced intro using the
                # ps_c banks (idle until the first mm2 in the bridge)
                pss = ps_cp.tile([P128, SC], f32, tag="ps_c", name="ps_c_t1")
                for hc in range(NH):
                    nc.tensor.matmul(
                        pss,
                        st[0]["odT"][1][:, hc, :],
                        st[0]["oeT"][ci][:, hc, :],
                        start=(hc == 0),
                        stop=(hc == NH - 1),
                    )
                nc.vector.reduce_max(
                    mx1[:, ci:ci + 1], pss, axis=mybir.AxisListType.X
                )
                ps1_hi[ci] = pss
        od_load(0, 2)
        prev0 = softmax_exp(0, 0, mx0, pss0)
        od_tr(0, 2)  # fills the PE wait on exp(0,c0) freeing ps_s0
        # tile 1 chunks 0/1 into the ps_s banks just freed by exp(0,c0/c1)
        pss1 = []
        mm1_chunk(0, 1, 0, mx1, pss1)
        mm1_chunk(0, 1, 1, mx1, pss1)
        pss1 += [ps1_hi[2], ps1_hi[3]]
        # exp(1) reads c2/c3 (ps_c banks) first so the bridge mm2's ps_c
        # slot reuse isn't stalled
        prev1 = softmax_exp(0, 1, mx1, pss1, order=(2, 3, 0, 1))
        # bridge: P^T(t0) + mm2(t0), plus od prep for the g=2 loop start
        pTs = ptr_stage(prev0)
        mm2(prev0, pTs)
        od_load(0, 3)
        prev = prev1

        # ---- steady state ----
        # next-batch oe pipeline: dma at iteration END of tt=4..11, DVE
        # adds at tt=5..12 (just after the P^T copies in the DVE FIFO, deps
        # already on-chip), PE transposes at tt=7..14.
        for g in range(2, BLOC * NT):
            b, tt = divmod(g, NT)
            # PE front: P^T(g-1) then MM1(g); the prep transposes go
            # after MM1 so their ps_tr slot reuse never stalls behind the
            # P^T PSUM->SBUF copies (the slots have drained by then)
            pTs = ptr_stage(prev)
            if b + 1 < BLOC and 5 <= tt < 13:
                oe_add(b + 1, 2 * (tt - 5))
                oe_add(b + 1, 2 * (tt - 5) + 1)
            mx, pss_list = mm1(b, tt)
            cur = softmax_exp(b, tt, mx, pss_list)  # ACT: exps before outscale
            if tt + 1 < NT:
                od_tr(b, tt + 1)
            elif b + 1 < BLOC:
                od_tr(b + 1, 0)
            if b + 1 < BLOC and 7 <= tt < 15:
                oe_tr(b + 1, 2 * (tt - 7))
                oe_tr(b + 1, 2 * (tt - 7) + 1)
            mm2(prev, pTs)
            prev = cur
            # iteration tail: DMA issues for future iterations
            if tt + 2 < NT:
                od_load(b, tt + 2)
            elif b + 1 < BLOC and tt + 2 - NT < 2:
                od_load(b + 1, tt + 2 - NT)
            if b + 1 < BLOC and 3 <= tt < 11:
                oe_dma(b + 1, 2 * (tt - 3))
                oe_dma(b + 1, 2 * (tt - 3) + 1)
        pTs = ptr_stage(prev)
        mm2(prev, pTs)

    nc.finalize()
    return nc


def _ensure_devices():
    """Make sure the 8 NeuronCores are visible to jax.devices().

    The calling harness may have pinned jax to cpu (JAX_PLATFORMS=cpu is a
    common pin for running the jax reference); the Bass SPMD launcher uses
    jax.devices(), so re-point jax at the neuron platform if needed.
    """
    import os
    import jax

    try:
        devs = jax.devices()
    except Exception:
        devs = []
    if sum(1 for d in devs if d.platform != "cpu") >= NCORES:
        return
    for plats in ("axon,cpu", None):
        try:
            if plats is None:
                os.environ.pop("JAX_PLATFORMS", None)
            else:
                os.environ["JAX_PLATFORMS"] = plats
            jax.config.update("jax_platforms", plats)
            from jax.extend.backend import clear_backends

            clear_backends()
            devs = jax.devices()
            if sum(1 for d in devs if d.platform != "cpu") >= NCORES:
                return
        except Exception:
            continue


def kernel(in_e=None, out_e=None, out_d=None, **kwargs):
    global _cached_nc
    from concourse.bass_utils import run_bass_kernel_spmd

    _ensure_devices()

    out_e = np.asarray(out_e, dtype=np.float32)
    out_d = np.asarray(out_d, dtype=np.float32)
    if _cached_nc is None:
        _cached_nc = _build()
    in_maps = []
    for c in range(NCORES):
        bsl = slice(c * BLOC, (c + 1) * BLOC)
        in_maps.append({
            "out_e": np.ascontiguousarray(out_e[:, bsl, :]),
            "out_d": np.ascontiguousarray(out_d[:, bsl, :]),
        })
    res = run_bass_kernel_spmd(_cached_nc, in_maps, list(range(NCORES)))
    return np.concatenate([res.results[c]["out"] for c in range(NCORES)], axis=1)



# revision 9
# speedup vs baseline: 1.1694x; 1.1694x over previous
"""Trainium2 Bass kernel for nn_Attention_43516608643501.

Cross-attention: Q = out_d [T,B,H]; K = V = sum of fwd/bwd halves of out_e
-> [S,B,H]; scores = Q @ K^T per batch (contraction over H, no scaling);
softmax over the source dim S; context = P @ V -> output [T,B,H].

Sharding: data-parallel over batch (dim 1): 2 batches per core x 8 cores,
no cross-core communication.

Design: one flattened software pipeline over 32 (batch, t-tile)
iterations.  Scores are computed in [t_partition, s_free] tiles so the
softmax max/sum are free-dim reductions (DVE reduce_max + ACT exp
accum_out).  P is transposed back to [s,t] on the PE (fp16, 1 cyc/row).
Per iteration g the PE queue is:
  [P-transposes(g-1)] [input-prep transposes] [MM1(g)] [MM2(g-1)]
so the PSUM->SBUF copies of P^T(g-1) (DVE) hide under MM1(g), and the
input-prep transposes' DMA+add dependencies were issued >=2 iterations
earlier.  exp(g) is queued on ACT before out-scale(g-1) so the strict
ACT FIFO can't delay MM1(g+1)'s PSUM-bank reuse.  Batch 1's input prep
is software-pipelined into batch 0's tail (oe DMAs at tiles 3..10, the
fwd+bwd adds -- on the otherwise idle GpSimd engine -- at 5..12, PE
transposes at 7..14), so the batch boundary has no pipeline bubble.
The od f32->f16 casts also run on GpSimd, keeping the DVE FIFO clear
for the P^T copies that gate MM2.

Numerics: both matmuls run in fp16 (full PE rate; fp16's 4.9e-4 rounding
vs bf16's 4e-3 matters because the scores carry no 1/sqrt(H) scaling, so
near-ties in the softmax amplify score error by exp()).

Intro/tail trimming: batch 0's DMAs are all issued up-front in
arrival-priority order striped over the sync+scalar rings (the DMA ramp
is power-capped, so first-needed bytes go first; the earliest tiles are
split across both rings).  The first half of batch 0's oe adds + the od0
cast run on the otherwise-idle DVE at ~2x GpSimd's rate, cutting the
arrival->consumable latency that paced the intro.  The final mm2 is
split into two h-halves so the last out-scale + store overlap the
closing matmuls.
"""

import numpy as np
from contextlib import ExitStack

S, T, B, H = 2048, 2048, 16, 512
NCORES = 8
BLOC = B // NCORES  # batches per core
P128 = 128
NS = S // P128  # 16 s-tiles
NT = T // P128  # 16 t-tiles
NH = H // P128  # 4 h-chunks of the contraction
SC = 512  # s-chunk width (scores tile columns)
NSC = S // SC  # 4 s-chunks per t-tile

_cached_nc = None


def _build():
    import concourse.bacc as bacc
    import concourse.tile as tile
    from concourse import mybir
    from concourse.masks import make_identity

    f32 = mybir.dt.float32
    f16 = mybir.dt.float16

    nc = bacc.Bacc(None, target_bir_lowering=False)
    d_oe = nc.dram_tensor("out_e", [S, BLOC, 2 * H], f32, kind="ExternalInput")
    d_od = nc.dram_tensor("out_d", [T, BLOC, H], f32, kind="ExternalInput")
    d_out = nc.dram_tensor("out", [T, BLOC, H], f32, kind="ExternalOutput")

    with ExitStack() as ctx:
        tc = ctx.enter_context(tile.TileContext(nc))
        singles = ctx.enter_context(tc.tile_pool(name="singles", bufs=1))
        loads = ctx.enter_context(tc.tile_pool(name="loads", bufs=8))
        persist = ctx.enter_context(tc.tile_pool(name="persist", bufs=2))
        work = ctx.enter_context(tc.tile_pool(name="work", bufs=4))
        ptile = ctx.enter_context(tc.tile_pool(name="ptile", bufs=2))
        outs = ctx.enter_context(tc.tile_pool(name="outs", bufs=3))
        small = ctx.enter_context(tc.tile_pool(name="small", bufs=3))
        # PSUM: 8 banks = ps_s0..3 (4) + tr (2) + ps_c (2)
        ps_s_pool = ctx.enter_context(tc.tile_pool(name="ps_s_pool", bufs=1, space="PSUM"))
        ps_tr = ctx.enter_context(tc.tile_pool(name="ps_tr", bufs=2, space="PSUM"))
        ps_cp = ctx.enter_context(tc.tile_pool(name="ps_cp", bufs=2, space="PSUM"))

        id16 = singles.tile([P128, P128], f16)
        make_identity(nc, id16)

        # per-batch persistent tile handles (persist pool tags rotate
        # bufs=2 slots, so consecutive batches double-buffer)
        st = [dict(oeT=[None] * NSC, odT=[None] * NT, nat=[None] * NS,
                   odf=[None] * NT) for _ in range(BLOC)]

        raws = {}

        def oe_dma(b, k, eng=None, split=False):
            raw = loads.tile([P128, 2 * H], f32, tag="raw", name="raw", bufs=10)
            src = d_oe[k * P128:(k + 1) * P128, b, :]
            if split:
                # halve across two hwdge rings so the first tiles finish
                # sooner in the power-capped DMA ramp
                nc.sync.dma_start(out=raw[:, 0:H], in_=src[:, 0:H])
                nc.scalar.dma_start(out=raw[:, H:2 * H], in_=src[:, H:2 * H])
            else:
                (eng or nc.sync).dma_start(out=raw, in_=src)
            raws[(b, k)] = raw

        def oe_add(b, k, eng=None):
            raw = raws.pop((b, k))
            nat = persist.tile([P128, H], f16, tag=f"oenat{k}", name=f"oenat{k}")
            (eng or nc.gpsimd).tensor_add(nat, raw[:, 0:H], raw[:, H:2 * H])
            st[b]["nat"][k] = nat

        def oe_load(b, k, eng=None):
            if (b, k) not in raws:
                oe_dma(b, k)
            oe_add(b, k, eng=eng)

        def oe_tr(b, k):
            nat = st[b]["nat"][k]
            ci, j = k // 4, k % 4
            if st[b]["oeT"][ci] is None or j == 0:
                st[b]["oeT"][ci] = persist.tile(
                    [P128, NH, SC], f16, tag=f"oeT{ci}", name=f"oeT{ci}"
                )
            trp = ps_tr.tile([P128, H], f16, tag="tr", name="tr_oe")
            for hc in range(NH):
                nc.tensor.transpose(
                    trp[:, hc * P128:(hc + 1) * P128],
                    nat[:, hc * P128:(hc + 1) * P128],
                    id16,
                )
            dst = st[b]["oeT"][ci][:, :, j * P128:(j + 1) * P128]
            src = trp.rearrange("p (h s) -> p h s", h=NH)
            nc.scalar.copy(dst, src)

        odrs = {}

        def od_dma(b, tt, eng=None, split=False):
            odr = loads.tile([P128, H], f32, tag="odr", name="odr")
            src = d_od[tt * P128:(tt + 1) * P128, b, :]
            if split:
                nc.sync.dma_start(out=odr[:, 0:H // 2], in_=src[:, 0:H // 2])
                nc.scalar.dma_start(out=odr[:, H // 2:H], in_=src[:, H // 2:H])
            else:
                (eng or nc.sync).dma_start(out=odr, in_=src)
            odrs[(b, tt)] = odr

        def od_cast(b, tt, eng=None):
            odr = odrs.pop((b, tt))
            odf = work.tile([P128, H], f16, tag="odf", name="odf")
            (eng or nc.gpsimd).tensor_copy(odf, odr)
            st[b]["odf"][tt] = odf

        def od_load(b, tt):
            od_dma(b, tt)
            od_cast(b, tt)

        def od_tr(b, tt):
            odf = st[b]["odf"][tt]
            trp = ps_tr.tile([P128, H], f16, tag="tr", name="tr_od")
            for hc in range(NH):
                nc.tensor.transpose(
                    trp[:, hc * P128:(hc + 1) * P128],
                    odf[:, hc * P128:(hc + 1) * P128],
                    id16,
                )
            odT = persist.tile([P128, NH, P128], f16, tag=f"odT{tt}", name=f"odT{tt}")
            nc.scalar.copy(odT, trp.rearrange("p (h t) -> p h t", h=NH))
            st[b]["odT"][tt] = odT

        def ptr_stage(prev):
            """PE transposes of P(g-1) [t,s]->[s,t] + DVE PSUM->SBUF copies."""
            _, _, pts, _ = prev
            pTs = []
            for half in range(2):
                ptr = ps_tr.tile([P128, 2, SC], f16, tag="tr", name="ptr")
                for sub in range(2):
                    ci = 2 * half + sub
                    for j in range(SC // P128):
                        nc.tensor.transpose(
                            ptr[:, sub, j * P128:(j + 1) * P128],
                            pts[ci][:, j * P128:(j + 1) * P128],
                            id16,
                        )
                pT = ptile.tile([P128, 2 * SC], f16, tag=f"pT{half}", name=f"pT{half}")
                nc.vector.tensor_copy(pT, ptr.rearrange("p a b -> p (a b)"))
                pTs.append(pT)
            return pTs

        def mm1_chunk(b, tt, ci, mx, pss_list):
            pss = ps_s_pool.tile([P128, SC], f32, tag=f"ps_s{ci}", name=f"ps_s{ci}")
            odT = st[b]["odT"][tt]
            oeT = st[b]["oeT"][ci]
            for hc in range(NH):
                nc.tensor.matmul(
                    pss,
                    odT[:, hc, :],
                    oeT[:, hc, :],
                    start=(hc == 0),
                    stop=(hc == NH - 1),
                )
            nc.vector.reduce_max(mx[:, ci:ci + 1], pss, axis=mybir.AxisListType.X)
            pss_list.append(pss)

        def mm1(b, tt):
            mx = small.tile([P128, NSC], f32, tag="mx", name="mx")
            pss_list = []
            for ci in range(NSC):
                mm1_chunk(b, tt, ci, mx, pss_list)
            return mx, pss_list

        def softmax_exp(b, tt, mx, pss_list, order=None):
            m = small.tile([P128, 1], f32, tag="m", name="m")
            nc.vector.reduce_max(m, mx, axis=mybir.AxisListType.X)
            neg_m = small.tile([P128, 1], f32, tag="neg_m", name="neg_m")
            nc.vector.tensor_scalar_mul(neg_m, m, -1.0)
            lacc = small.tile([P128, NSC], f32, tag="lacc", name="lacc")
            pts = [None] * NSC
            for ci in (order if order is not None else range(NSC)):
                pt = ptile.tile([P128, SC], f16, tag=f"pt{ci}", name=f"pt{ci}")
                nc.scalar.activation(
                    pt, pss_list[ci],
                    mybir.ActivationFunctionType.Exp,
                    bias=neg_m, scale=1.0,
                    accum_out=lacc[:, ci:ci + 1],
                )
                pts[ci] = pt
            l = small.tile([P128, 1], f32, tag="l", name="l")
            nc.vector.reduce_sum(l, lacc, axis=mybir.AxisListType.X)
            linv = small.tile([P128, 1], f32, tag="linv", name="linv")
            nc.vector.reciprocal(linv, l)
            return b, tt, pts, linv

        def mm2_mms(prev, pTs):
            pb, _, _, _ = prev
            ps_c = ps_cp.tile([P128, H], f32, tag="ps_c", name="ps_c")
            nat = st[pb]["nat"]
            for k in range(NS):
                nc.tensor.matmul(
                    ps_c,
                    pTs[k // 8][:, (k % 8) * P128:(k % 8 + 1) * P128],
                    nat[k],
                    start=(k == 0), stop=(k == NS - 1),
                )
            return ps_c

        def mm2_out(prev, ps_c):
            pb, ptt, _, plinv = prev
            ot = outs.tile([P128, H], f32, tag="ot", name="ot")
            nc.scalar.activation(
                ot, ps_c, mybir.ActivationFunctionType.Identity,
                bias=0.0, scale=plinv,
            )
            nc.sync.dma_start(
                out=d_out[ptt * P128:(ptt + 1) * P128, pb, :], in_=ot
            )

        def mm2(prev, pTs):
            mm2_out(prev, mm2_mms(prev, pTs))

        # ---- intro: batch 0 tile 0, chunk-interleaved with the oe loads
        # (DMA-bound; the PE stalls here are unavoidable) ----
        # All of batch 0's DMAs are issued up-front in arrival-priority
        # order, striped over the sync+scalar hwdge rings: the DMA ramp is
        # power-capped, so the binding resource is aggregate bytes -- put
        # the first-needed tiles first and split them across both rings so
        # they complete earliest.  od1/od2 ride behind the oe tiles they
        # won't be needed before.  (Ring entries for k>=10 carry raw-slot
        # WAR waits; by then arrivals are BW-bound anyway.)
        oe_dma(0, 0, split=True)
        oe_dma(0, 1, split=True)
        od_dma(0, 0, split=True)
        for k in range(2, 8):
            oe_dma(0, k, eng=(nc.scalar if k % 2 else nc.sync))
        od_dma(0, 1, eng=nc.sync)
        for k in range(8, 14):
            oe_dma(0, k, eng=(nc.scalar if k % 2 else nc.sync))
        od_dma(0, 2, eng=nc.scalar)
        oe_dma(0, 14, eng=nc.sync)
        oe_dma(0, 15, eng=nc.scalar)
        # intro adds/casts split across DVE (first half -- DVE is idle
        # until the first reduce_max, and at ~650ns/op halves the
        # arrival->consumable latency vs GpSimd's 1.3-2.2us) and GpSimd
        # (second half, once arrivals pace out and DVE picks up softmax
        # work for t0/t1)
        od_cast(0, 0, eng=nc.vector)
        mx0 = small.tile([P128, NSC], f32, tag="mx", name="mx")
        mx1 = small.tile([P128, NSC], f32, tag="mx", name="mx1")
        pss0 = []
        ps1_hi = {}
        HC2 = SC // 2
        for ci in range(NSC):
            if ci < 2:
                # chunks 0/1 land in the fully DMA-starved region: run
                # N=256 sub-matmuls per 2-tile arrival so the PE starts
                # (and HAM warms) as early as possible
                pss = ps_s_pool.tile(
                    [P128, SC], f32, tag=f"ps_s{ci}", name=f"ps_s{ci}"
                )
                for j in range(2):
                    for k in range(4 * ci + 2 * j, 4 * ci + 2 * j + 2):
                        oe_load(0, k, eng=nc.vector)
                        oe_tr(0, k)
                    if ci == 0 and j == 0:
                        od_tr(0, 0)
                    for hc in range(NH):
                        nc.tensor.matmul(
                            pss[:, j * HC2:(j + 1) * HC2],
                            st[0]["odT"][0][:, hc, :],
                            st[0]["oeT"][ci][:, hc, j * HC2:(j + 1) * HC2],
                            start=(hc == 0),
                            stop=(hc == NH - 1),
                            skip_group_check=True,
                        )
                nc.vector.reduce_max(
                    mx0[:, ci:ci + 1], pss, axis=mybir.AxisListType.X
                )
                pss0.append(pss)
                if ci == 1:
                    od_cast(0, 1)  # GpSimd: idle from here to b1 prefetch
                    od_tr(0, 1)
                continue
            for k in range(4 * ci, 4 * ci + 4):
                oe_load(0, k)
                oe_tr(0, k)
            mm1_chunk(0, 0, ci, mx0, pss0)
            if ci >= 2:
                # tile 1's chunks 2/3 fill the DMA-paced intro using the
                # ps_c banks (idle until the first mm2 in the bridge)
                pss = ps_cp.tile([P128, SC], f32, tag="ps_c", name="ps_c_t1")
                for hc in range(NH):
                    nc.tensor.matmul(
                        pss,
                        st[0]["odT"][1][:, hc, :],
                        st[0]["oeT"][ci][:, hc, :],
                        start=(hc == 0),
                        stop=(hc == NH - 1),
                    )
                nc.vector.reduce_max(
                    mx1[:, ci:ci + 1], pss, axis=mybir.AxisListType.X
                )
                ps1_hi[ci] = pss
        od_cast(0, 2)
        prev0 = softmax_exp(0, 0, mx0, pss0)
        od_tr(0, 2)  # fills the PE wait on exp(0,c0) freeing ps_s0
        # tile 1 chunks 0/1 into the ps_s banks just freed by exp(0,c0/c1)
        pss1 = []
        mm1_chunk(0, 1, 0, mx1, pss1)
        mm1_chunk(0, 1, 1, mx1, pss1)
        pss1 += [ps1_hi[2], ps1_hi[3]]
        # exp(1) reads c2/c3 (ps_c banks) first so the bridge mm2's ps_c
        # slot reuse isn't stalled
        prev1 = softmax_exp(0, 1, mx1, pss1, order=(2, 3, 0, 1))
        # bridge: P^T(t0) + mm2(t0), plus od prep for the g=2 loop start
        pTs = ptr_stage(prev0)
        mm2(prev0, pTs)
        od_load(0, 3)
        prev = prev1

        # ---- steady state ----
        # next-batch oe pipeline: dma at iteration END of tt=4..11, DVE
        # adds at tt=5..12 (just after the P^T copies in the DVE FIFO, deps
        # already on-chip), PE transposes at tt=7..14.
        for g in range(2, BLOC * NT):
            b, tt = divmod(g, NT)
            # PE front: P^T(g-1) then MM1(g); the prep transposes go
            # after MM1 so their ps_tr slot reuse never stalls behind the
            # P^T PSUM->SBUF copies (the slots have drained by then)
            pTs = ptr_stage(prev)
            if b + 1 < BLOC and 5 <= tt < 13:
                oe_add(b + 1, 2 * (tt - 5))
                oe_add(b + 1, 2 * (tt - 5) + 1)
            mx, pss_list = mm1(b, tt)
            cur = softmax_exp(b, tt, mx, pss_list)  # ACT: exps before outscale
            if tt + 1 < NT:
                od_tr(b, tt + 1)
            elif b + 1 < BLOC:
                od_tr(b + 1, 0)
            if b + 1 < BLOC and 7 <= tt < 15:
                oe_tr(b + 1, 2 * (tt - 7))
                oe_tr(b + 1, 2 * (tt - 7) + 1)
            mm2(prev, pTs)
            prev = cur
            # iteration tail: DMA issues for future iterations
            if tt + 2 < NT:
                od_load(b, tt + 2)
            elif b + 1 < BLOC and tt + 2 - NT < 2:
                od_load(b + 1, tt + 2 - NT)
            if b + 1 < BLOC and 3 <= tt < 11:
                oe_dma(b + 1, 2 * (tt - 3))
                oe_dma(b + 1, 2 * (tt - 3) + 1)
        # final iteration: split mm2 into two h-halves so the first half's
        # out-scale + store DMA overlap the second half's matmuls instead
        # of serializing after the very last PE op
        pTs = ptr_stage(prev)
        pb, ptt, _, plinv = prev
        H2 = H // 2
        nat = st[pb]["nat"]
        for half in range(2):
            ps = ps_cp.tile([P128, H2], f32, tag="ps_c", name="ps_c_tail")
            for k in range(NS):
                nc.tensor.matmul(
                    ps,
                    pTs[k // 8][:, (k % 8) * P128:(k % 8 + 1) * P128],
                    nat[k][:, half * H2:(half + 1) * H2],
                    start=(k == 0), stop=(k == NS - 1),
                )
            ot = outs.tile([P128, H2], f32, tag=f"ot_tail{half}", name=f"ot_tail{half}")
            nc.scalar.activation(
                ot, ps, mybir.ActivationFunctionType.Identity,
                bias=0.0, scale=plinv,
            )
            nc.sync.dma_start(
                out=d_out[ptt * P128:(ptt + 1) * P128, pb,
                          half * H2:(half + 1) * H2],
                in_=ot,
            )

    nc.finalize()
    return nc


def _ensure_devices():
    """Make sure the 8 NeuronCores are visible to jax.devices().

    The calling harness may have pinned jax to cpu (JAX_PLATFORMS=cpu is a
    common pin for running the jax reference); the Bass SPMD launcher uses
    jax.devices(), so re-point jax at the neuron platform if needed.
    """
    import os
    import jax

    try:
        devs = jax.devices()
    except Exception:
        devs = []
    if sum(1 for d in devs if d.platform != "cpu") >= NCORES:
        return
    for plats in ("axon,cpu", None):
        try:
            if plats is None:
                os.environ.pop("JAX_PLATFORMS", None)
            else:
                os.environ["JAX_PLATFORMS"] = plats
            jax.config.update("jax_platforms", plats)
            from jax.extend.backend import clear_backends

            clear_backends()
            devs = jax.devices()
            if sum(1 for d in devs if d.platform != "cpu") >= NCORES:
                return
        except Exception:
            continue


def kernel(in_e=None, out_e=None, out_d=None, **kwargs):
    global _cached_nc
    from concourse.bass_utils import run_bass_kernel_spmd

    _ensure_devices()

    out_e = np.asarray(out_e, dtype=np.float32)
    out_d = np.asarray(out_d, dtype=np.float32)
    if _cached_nc is None:
        _cached_nc = _build()
    in_maps = []
    for c in range(NCORES):
        bsl = slice(c * BLOC, (c + 1) * BLOC)
        in_maps.append({
            "out_e": np.ascontiguousarray(out_e[:, bsl, :]),
            "out_d": np.ascontiguousarray(out_d[:, bsl, :]),
        })
    res = run_bass_kernel_spmd(_cached_nc, in_maps, list(range(NCORES)))
    return np.concatenate([res.results[c]["out"] for c in range(NCORES)], axis=1)



# revision 13
# speedup vs baseline: 1.1830x; 1.0116x over previous
"""Trainium2 Bass kernel for nn_Attention_43516608643501.

Cross-attention: Q = out_d [T,B,H]; K = V = sum of fwd/bwd halves of out_e
-> [S,B,H]; scores = Q @ K^T per batch (contraction over H, no scaling);
softmax over the source dim S; context = P @ V -> output [T,B,H].

Sharding: data-parallel over batch (dim 1): 2 batches per core x 8 cores,
no cross-core communication.

Design: one flattened software pipeline over 32 (batch, t-tile)
iterations.  Scores are computed in [t_partition, s_free] tiles so the
softmax max/sum are free-dim reductions (DVE reduce_max + ACT exp
accum_out).  P is transposed back to [s,t] on the PE (fp16, 1 cyc/row).
Per iteration g the PE queue is:
  [P-transposes(g-1)] [input-prep transposes] [MM1(g)] [MM2(g-1)]
so the PSUM->SBUF copies of P^T(g-1) (DVE) hide under MM1(g), and the
input-prep transposes' DMA+add dependencies were issued >=2 iterations
earlier.  exp(g) is queued on ACT before out-scale(g-1) so the strict
ACT FIFO can't delay MM1(g+1)'s PSUM-bank reuse.  Batch 1's input prep
is software-pipelined into batch 0's tail (oe DMAs at tiles 3..10, the
fwd+bwd adds -- on the otherwise idle GpSimd engine -- at 5..12, PE
transposes at 7..14), so the batch boundary has no pipeline bubble.
The od f32->f16 casts also run on GpSimd, keeping the DVE FIFO clear
for the P^T copies that gate MM2.

Numerics: both matmuls run in fp16 (full PE rate; fp16's 4.9e-4 rounding
vs bf16's 4e-3 matters because the scores carry no 1/sqrt(H) scaling, so
near-ties in the softmax amplify score error by exp()).

Intro/tail trimming: batch 0's DMAs are all issued up-front in
arrival-priority order striped over the sync+scalar rings (the DMA ramp
is power-capped, so first-needed bytes go first; the earliest tiles are
split across both rings).  The first half of batch 0's oe adds + the od0
cast run on the otherwise-idle DVE at ~2x GpSimd's rate, cutting the
arrival->consumable latency that paced the intro.  The final mm2 is
split into two h-halves so the last out-scale + store overlap the
closing matmuls.
"""

import numpy as np
from contextlib import ExitStack

S, T, B, H = 2048, 2048, 16, 512
NCORES = 8
BLOC = B // NCORES  # batches per core
P128 = 128
NS = S // P128  # 16 s-tiles
NT = T // P128  # 16 t-tiles
NH = H // P128  # 4 h-chunks of the contraction
SC = 512  # s-chunk width (scores tile columns)
NSC = S // SC  # 4 s-chunks per t-tile

_cached_nc = None


def _build():
    import concourse.bacc as bacc
    import concourse.tile as tile
    from concourse import mybir
    from concourse.masks import make_identity

    f32 = mybir.dt.float32
    f16 = mybir.dt.float16

    nc = bacc.Bacc(None, target_bir_lowering=False)
    d_oe = nc.dram_tensor("out_e", [S, BLOC, 2 * H], f32, kind="ExternalInput")
    d_od = nc.dram_tensor("out_d", [T, BLOC, H], f32, kind="ExternalInput")
    d_out = nc.dram_tensor("out", [T, BLOC, H], f32, kind="ExternalOutput")

    with ExitStack() as ctx:
        tc = ctx.enter_context(tile.TileContext(nc))
        singles = ctx.enter_context(tc.tile_pool(name="singles", bufs=1))
        loads = ctx.enter_context(tc.tile_pool(name="loads", bufs=8))
        persist = ctx.enter_context(tc.tile_pool(name="persist", bufs=2))
        work = ctx.enter_context(tc.tile_pool(name="work", bufs=4))
        ptile = ctx.enter_context(tc.tile_pool(name="ptile", bufs=2))
        outs = ctx.enter_context(tc.tile_pool(name="outs", bufs=3))
        small = ctx.enter_context(tc.tile_pool(name="small", bufs=3))
        # PSUM: 8 banks = ps_s0..3 (4) + tr (2) + ps_c (2)
        ps_s_pool = ctx.enter_context(tc.tile_pool(name="ps_s_pool", bufs=1, space="PSUM"))
        ps_tr = ctx.enter_context(tc.tile_pool(name="ps_tr", bufs=2, space="PSUM"))
        ps_cp = ctx.enter_context(tc.tile_pool(name="ps_cp", bufs=2, space="PSUM"))

        id16 = singles.tile([P128, P128], f16)
        make_identity(nc, id16)

        # per-batch persistent tile handles (persist pool tags rotate
        # bufs=2 slots, so consecutive batches double-buffer)
        st = [dict(oeT=[None] * NSC, odT=[None] * NT, nat=[None] * NS,
                   odf=[None] * NT) for _ in range(BLOC)]

        raws = {}

        def oe_dma(b, k, eng=None, split=False):
            raw = loads.tile([P128, 2 * H], f32, tag="raw", name="raw", bufs=10)
            src = d_oe[k * P128:(k + 1) * P128, b, :]
            if split:
                # halve across two hwdge rings so the first tiles finish
                # sooner in the power-capped DMA ramp
                nc.sync.dma_start(out=raw[:, 0:H], in_=src[:, 0:H])
                nc.scalar.dma_start(out=raw[:, H:2 * H], in_=src[:, H:2 * H])
            else:
                (eng or nc.sync).dma_start(out=raw, in_=src)
            raws[(b, k)] = raw

        def oe_add(b, k, eng=None):
            raw = raws.pop((b, k))
            nat = persist.tile([P128, H], f16, tag=f"oenat{k}", name=f"oenat{k}")
            (eng or nc.gpsimd).tensor_add(nat, raw[:, 0:H], raw[:, H:2 * H])
            st[b]["nat"][k] = nat

        def oe_load(b, k, eng=None):
            if (b, k) not in raws:
                oe_dma(b, k)
            oe_add(b, k, eng=eng)

        def oe_tr(b, k):
            nat = st[b]["nat"][k]
            ci, j = k // 4, k % 4
            if st[b]["oeT"][ci] is None or j == 0:
                st[b]["oeT"][ci] = persist.tile(
                    [P128, NH, SC], f16, tag=f"oeT{ci}", name=f"oeT{ci}"
                )
            trp = ps_tr.tile([P128, H], f16, tag="tr", name="tr_oe")
            for hc in range(NH):
                nc.tensor.transpose(
                    trp[:, hc * P128:(hc + 1) * P128],
                    nat[:, hc * P128:(hc + 1) * P128],
                    id16,
                )
            dst = st[b]["oeT"][ci][:, :, j * P128:(j + 1) * P128]
            src = trp.rearrange("p (h s) -> p h s", h=NH)
            nc.scalar.copy(dst, src)

        odrs = {}

        def od_dma(b, tt, eng=None, split=False):
            odr = loads.tile([P128, H], f32, tag="odr", name="odr")
            src = d_od[tt * P128:(tt + 1) * P128, b, :]
            if split:
                nc.sync.dma_start(out=odr[:, 0:H // 2], in_=src[:, 0:H // 2])
                nc.scalar.dma_start(out=odr[:, H // 2:H], in_=src[:, H // 2:H])
            else:
                (eng or nc.sync).dma_start(out=odr, in_=src)
            odrs[(b, tt)] = odr

        def od_cast(b, tt, eng=None):
            odr = odrs.pop((b, tt))
            odf = work.tile([P128, H], f16, tag="odf", name="odf")
            (eng or nc.gpsimd).tensor_copy(odf, odr)
            st[b]["odf"][tt] = odf

        def od_load(b, tt):
            od_dma(b, tt)
            od_cast(b, tt)

        def od_tr(b, tt):
            odf = st[b]["odf"][tt]
            trp = ps_tr.tile([P128, H], f16, tag="tr", name="tr_od")
            for hc in range(NH):
                nc.tensor.transpose(
                    trp[:, hc * P128:(hc + 1) * P128],
                    odf[:, hc * P128:(hc + 1) * P128],
                    id16,
                )
            odT = persist.tile([P128, NH, P128], f16, tag=f"odT{tt}", name=f"odT{tt}")
            nc.scalar.copy(odT, trp.rearrange("p (h t) -> p h t", h=NH))
            st[b]["odT"][tt] = odT

        def ptr_stage(prev):
            """PE transposes of P(g-1) [t,s]->[s,t] + DVE PSUM->SBUF copies."""
            _, _, pts, _ = prev
            pTs = []
            for half in range(2):
                ptr = ps_tr.tile([P128, 2, SC], f16, tag="tr", name="ptr")
                for sub in range(2):
                    ci = 2 * half + sub
                    for j in range(SC // P128):
                        nc.tensor.transpose(
                            ptr[:, sub, j * P128:(j + 1) * P128],
                            pts[ci][:, j * P128:(j + 1) * P128],
                            id16,
                        )
                pT = ptile.tile([P128, 2 * SC], f16, tag=f"pT{half}", name=f"pT{half}")
                nc.vector.tensor_copy(pT, ptr.rearrange("p a b -> p (a b)"))
                pTs.append(pT)
            return pTs

        def mm1_chunk(b, tt, ci, mx, pss_list):
            pss = ps_s_pool.tile([P128, SC], f32, tag=f"ps_s{ci}", name=f"ps_s{ci}")
            odT = st[b]["odT"][tt]
            oeT = st[b]["oeT"][ci]
            for hc in range(NH):
                nc.tensor.matmul(
                    pss,
                    odT[:, hc, :],
                    oeT[:, hc, :],
                    start=(hc == 0),
                    stop=(hc == NH - 1),
                )
            nc.vector.reduce_max(mx[:, ci:ci + 1], pss, axis=mybir.AxisListType.X)
            pss_list.append(pss)

        def mm1(b, tt):
            mx = small.tile([P128, NSC], f32, tag="mx", name="mx")
            pss_list = []
            for ci in range(NSC):
                mm1_chunk(b, tt, ci, mx, pss_list)
            return mx, pss_list

        def softmax_exp(b, tt, mx, pss_list, order=None):
            m = small.tile([P128, 1], f32, tag="m", name="m")
            nc.vector.reduce_max(m, mx, axis=mybir.AxisListType.X)
            neg_m = small.tile([P128, 1], f32, tag="neg_m", name="neg_m")
            nc.vector.tensor_scalar_mul(neg_m, m, -1.0)
            lacc = small.tile([P128, NSC], f32, tag="lacc", name="lacc")
            pts = [None] * NSC
            for ci in (order if order is not None else range(NSC)):
                pt = ptile.tile([P128, SC], f16, tag=f"pt{ci}", name=f"pt{ci}")
                nc.scalar.activation(
                    pt, pss_list[ci],
                    mybir.ActivationFunctionType.Exp,
                    bias=neg_m, scale=1.0,
                    accum_out=lacc[:, ci:ci + 1],
                )
                pts[ci] = pt
            l = small.tile([P128, 1], f32, tag="l", name="l")
            nc.vector.reduce_sum(l, lacc, axis=mybir.AxisListType.X)
            linv = small.tile([P128, 1], f32, tag="linv", name="linv")
            nc.vector.reciprocal(linv, l)
            return b, tt, pts, linv

        def mm2_mms(prev, pTs):
            pb, _, _, _ = prev
            ps_c = ps_cp.tile([P128, H], f32, tag="ps_c", name="ps_c")
            nat = st[pb]["nat"]
            for k in range(NS):
                nc.tensor.matmul(
                    ps_c,
                    pTs[k // 8][:, (k % 8) * P128:(k % 8 + 1) * P128],
                    nat[k],
                    start=(k == 0), stop=(k == NS - 1),
                )
            return ps_c

        def mm2_out(prev, ps_c):
            pb, ptt, _, plinv = prev
            ot = outs.tile([P128, H], f32, tag="ot", name="ot")
            nc.scalar.activation(
                ot, ps_c, mybir.ActivationFunctionType.Identity,
                bias=0.0, scale=plinv,
            )
            nc.sync.dma_start(
                out=d_out[ptt * P128:(ptt + 1) * P128, pb, :], in_=ot
            )

        def mm2(prev, pTs):
            mm2_out(prev, mm2_mms(prev, pTs))

        # ---- intro: batch 0 tile 0, chunk-interleaved with the oe loads
        # (DMA-bound; the PE stalls here are unavoidable) ----
        # All of batch 0's DMAs are issued up-front in arrival-priority
        # order, striped over the sync+scalar hwdge rings: the DMA ramp is
        # power-capped, so the binding resource is aggregate bytes -- put
        # the first-needed tiles first and split them across both rings so
        # they complete earliest.  od1/od2 ride behind the oe tiles they
        # won't be needed before.  (Ring entries for k>=10 carry raw-slot
        # WAR waits; by then arrivals are BW-bound anyway.)
        # two balanced independent streams; per-tile arrivals interleave
        # ~1.5us apart so the PE is fed steadily (a strict priority order
        # on both rings clumps arrivals and starves the PE mid-intro)
        oe_dma(0, 0, split=True)
        od_dma(0, 0, eng=nc.sync)
        oe_dma(0, 1, eng=nc.scalar)
        oe_dma(0, 2, eng=nc.sync)
        oe_dma(0, 3, eng=nc.scalar)
        oe_dma(0, 4, eng=nc.sync)
        oe_dma(0, 5, eng=nc.scalar)
        od_dma(0, 1, eng=nc.sync)
        oe_dma(0, 6, eng=nc.sync)
        oe_dma(0, 7, eng=nc.scalar)
        oe_dma(0, 8, eng=nc.sync)
        oe_dma(0, 9, eng=nc.scalar)
        oe_dma(0, 10, eng=nc.sync)
        oe_dma(0, 11, eng=nc.scalar)
        oe_dma(0, 12, eng=nc.sync)
        oe_dma(0, 13, eng=nc.scalar)
        od_dma(0, 2, eng=nc.scalar)
        oe_dma(0, 14, eng=nc.sync)
        oe_dma(0, 15, eng=nc.scalar)
        # intro adds/casts split across DVE (first half -- DVE is idle
        # until the first reduce_max, and at ~650ns/op halves the
        # arrival->consumable latency vs GpSimd's 1.3-2.2us) and GpSimd
        # (second half, once arrivals pace out and DVE picks up softmax
        # work for t0/t1)
        od_cast(0, 0, eng=nc.vector)
        mx0 = small.tile([P128, NSC], f32, tag="mx", name="mx")
        mx1 = small.tile([P128, NSC], f32, tag="mx", name="mx1")
        pss0 = []
        ps1_hi = {}
        HC2 = SC // 2
        for ci in range(NSC):
            if ci < 2:
                # chunks 0/1 land in the fully DMA-starved region: run
                # N=256 sub-matmuls per 2-tile arrival so the PE starts
                # (and HAM warms) as early as possible
                pss = ps_s_pool.tile(
                    [P128, SC], f32, tag=f"ps_s{ci}", name=f"ps_s{ci}"
                )
                for j in range(2):
                    for k in range(4 * ci + 2 * j, 4 * ci + 2 * j + 2):
                        oe_load(0, k, eng=nc.vector)
                        oe_tr(0, k)
                    if ci == 0 and j == 0:
                        od_tr(0, 0)
                    if ci == 1 and j == 0:
                        # DVE slot between add k5 and add k6 (od1 lands
                        # between tiles 5 and 6 on the sync stream)
                        od_cast(0, 1, eng=nc.vector)
                    for hc in range(NH):
                        nc.tensor.matmul(
                            pss[:, j * HC2:(j + 1) * HC2],
                            st[0]["odT"][0][:, hc, :],
                            st[0]["oeT"][ci][:, hc, j * HC2:(j + 1) * HC2],
                            start=(hc == 0),
                            stop=(hc == NH - 1),
                            skip_group_check=True,
                        )
                nc.vector.reduce_max(
                    mx0[:, ci:ci + 1], pss, axis=mybir.AxisListType.X
                )
                pss0.append(pss)
                if ci == 1:
                    od_tr(0, 1)
                continue
            for k in range(4 * ci, 4 * ci + 4):
                oe_load(0, k)
                oe_tr(0, k)
            mm1_chunk(0, 0, ci, mx0, pss0)
            if ci >= 2:
                # tile 1's chunks 2/3 fill the DMA-paced intro using the
                # ps_c banks (idle until the first mm2 in the bridge)
                pss = ps_cp.tile([P128, SC], f32, tag="ps_c", name="ps_c_t1")
                for hc in range(NH):
                    nc.tensor.matmul(
                        pss,
                        st[0]["odT"][1][:, hc, :],
                        st[0]["oeT"][ci][:, hc, :],
                        start=(hc == 0),
                        stop=(hc == NH - 1),
                    )
                nc.vector.reduce_max(
                    mx1[:, ci:ci + 1], pss, axis=mybir.AxisListType.X
                )
                ps1_hi[ci] = pss
        od_cast(0, 2, eng=nc.vector)
        prev0 = softmax_exp(0, 0, mx0, pss0)
        od_tr(0, 2)  # fills the PE wait on exp(0,c0) freeing ps_s0
        # tile 1 chunks 0/1 into the ps_s banks just freed by exp(0,c0/c1)
        pss1 = []
        mm1_chunk(0, 1, 0, mx1, pss1)
        mm1_chunk(0, 1, 1, mx1, pss1)
        pss1 += [ps1_hi[2], ps1_hi[3]]
        # exp(1) reads c2/c3 (ps_c banks) first so the bridge mm2's ps_c
        # slot reuse isn't stalled
        prev1 = softmax_exp(0, 1, mx1, pss1, order=(2, 3, 0, 1))
        # bridge: P^T(t0) + mm2(t0), plus od prep for the g=2 loop start
        pTs = ptr_stage(prev0)
        mm2(prev0, pTs)
        od_load(0, 3)
        prev = prev1

        # ---- steady state ----
        # next-batch oe pipeline: dma at iteration END of tt=4..11, DVE
        # adds at tt=5..12 (just after the P^T copies in the DVE FIFO, deps
        # already on-chip), PE transposes at tt=7..14.
        for g in range(2, BLOC * NT):
            b, tt = divmod(g, NT)
            # PE front: P^T(g-1) then MM1(g); the prep transposes go
            # after MM1 so their ps_tr slot reuse never stalls behind the
            # P^T PSUM->SBUF copies (the slots have drained by then)
            pTs = ptr_stage(prev)
            if b + 1 < BLOC and 5 <= tt < 13:
                oe_add(b + 1, 2 * (tt - 5))
                oe_add(b + 1, 2 * (tt - 5) + 1)
            mx, pss_list = mm1(b, tt)
            cur = softmax_exp(b, tt, mx, pss_list)  # ACT: exps before outscale
            if tt + 1 < NT:
                od_tr(b, tt + 1)
            elif b + 1 < BLOC:
                od_tr(b + 1, 0)
            if b + 1 < BLOC and 7 <= tt < 15:
                oe_tr(b + 1, 2 * (tt - 7))
                oe_tr(b + 1, 2 * (tt - 7) + 1)
            mm2(prev, pTs)
            prev = cur
            # iteration tail: DMA issues for future iterations
            if tt + 2 < NT:
                od_load(b, tt + 2)
            elif b + 1 < BLOC and tt + 2 - NT < 2:
                od_load(b + 1, tt + 2 - NT)
            if b + 1 < BLOC and 3 <= tt < 11:
                oe_dma(b + 1, 2 * (tt - 3))
                oe_dma(b + 1, 2 * (tt - 3) + 1)
        # final iteration: split mm2 into two h-halves so the first half's
        # out-scale + store DMA overlap the second half's matmuls instead
        # of serializing after the very last PE op
        pTs = ptr_stage(prev)
        pb, ptt, _, plinv = prev
        H2 = H // 2
        nat = st[pb]["nat"]
        for half in range(2):
            ps = ps_cp.tile([P128, H2], f32, tag="ps_c", name="ps_c_tail")
            for k in range(NS):
                nc.tensor.matmul(
                    ps,
                    pTs[k // 8][:, (k % 8) * P128:(k % 8 + 1) * P128],
                    nat[k][:, half * H2:(half + 1) * H2],
                    start=(k == 0), stop=(k == NS - 1),
                )
            ot = outs.tile([P128, H2], f32, tag=f"ot_tail{half}", name=f"ot_tail{half}")
            nc.scalar.activation(
                ot, ps, mybir.ActivationFunctionType.Identity,
                bias=0.0, scale=plinv,
            )
            nc.sync.dma_start(
                out=d_out[ptt * P128:(ptt + 1) * P128, pb,
                          half * H2:(half + 1) * H2],
                in_=ot,
            )

    nc.finalize()
    return nc


def _ensure_devices():
    """Make sure the 8 NeuronCores are visible to jax.devices().

    The calling harness may have pinned jax to cpu (JAX_PLATFORMS=cpu is a
    common pin for running the jax reference); the Bass SPMD launcher uses
    jax.devices(), so re-point jax at the neuron platform if needed.
    """
    import os
    import jax

    try:
        devs = jax.devices()
    except Exception:
        devs = []
    if sum(1 for d in devs if d.platform != "cpu") >= NCORES:
        return
    for plats in ("axon,cpu", None):
        try:
            if plats is None:
                os.environ.pop("JAX_PLATFORMS", None)
            else:
                os.environ["JAX_PLATFORMS"] = plats
            jax.config.update("jax_platforms", plats)
            from jax.extend.backend import clear_backends

            clear_backends()
            devs = jax.devices()
            if sum(1 for d in devs if d.platform != "cpu") >= NCORES:
                return
        except Exception:
            continue


def kernel(in_e=None, out_e=None, out_d=None, **kwargs):
    global _cached_nc
    from concourse.bass_utils import run_bass_kernel_spmd

    _ensure_devices()

    out_e = np.asarray(out_e, dtype=np.float32)
    out_d = np.asarray(out_d, dtype=np.float32)
    if _cached_nc is None:
        _cached_nc = _build()
    in_maps = []
    for c in range(NCORES):
        bsl = slice(c * BLOC, (c + 1) * BLOC)
        in_maps.append({
            "out_e": np.ascontiguousarray(out_e[:, bsl, :]),
            "out_d": np.ascontiguousarray(out_d[:, bsl, :]),
        })
    res = run_bass_kernel_spmd(_cached_nc, in_maps, list(range(NCORES)))
    return np.concatenate([res.results[c]["out"] for c in range(NCORES)], axis=1)



# revision 23
# speedup vs baseline: 1.1980x; 1.0127x over previous
"""Trainium2 Bass kernel for nn_Attention_43516608643501.

Cross-attention: Q = out_d [T,B,H]; K = V = sum of fwd/bwd halves of out_e
-> [S,B,H]; scores = Q @ K^T per batch (contraction over H, no scaling);
softmax over the source dim S; context = P @ V -> output [T,B,H].

Sharding: data-parallel over batch (dim 1): 2 batches per core x 8 cores,
no cross-core communication.

Design: one flattened software pipeline over 32 (batch, t-tile)
iterations.  Scores are computed in [t_partition, s_free] tiles so the
softmax max/sum are free-dim reductions (DVE reduce_max + ACT exp
accum_out).  P is transposed back to [s,t] on the PE (fp16, 1 cyc/row).
Per iteration g the PE queue is:
  [P-transposes(g-1)] [input-prep transposes] [MM1(g)] [MM2(g-1)]
so the PSUM->SBUF copies of P^T(g-1) (DVE) hide under MM1(g), and the
input-prep transposes' DMA+add dependencies were issued >=2 iterations
earlier.  exp(g) is queued on ACT before out-scale(g-1) so the strict
ACT FIFO can't delay MM1(g+1)'s PSUM-bank reuse.  Batch 1's input prep
is software-pipelined into batch 0's tail (oe DMAs at tiles 3..10, the
fwd+bwd adds -- on the otherwise idle GpSimd engine -- at 5..12, PE
transposes at 7..14), so the batch boundary has no pipeline bubble.
The od f32->f16 casts also run on GpSimd, keeping the DVE FIFO clear
for the P^T copies that gate MM2.

Numerics: both matmuls run in fp16 (full PE rate; fp16's 4.9e-4 rounding
vs bf16's 4e-3 matters because the scores carry no 1/sqrt(H) scaling, so
near-ties in the softmax amplify score error by exp()).

Optimization notes (this session): the kernel is at the PE roofline for
this formulation -- 622592 PE rows (GEMMs 524288 + P^T 65536 + input
transposes 32768) stream back-to-back at the device's sustained clock
with <4% idle.  Measured per-process clock modes put best-case runs at
~302-305us.  Intro reshuffles (DMA ring striping, DVE-assisted adds,
consumption-order delivery) and a split final mm2 were each benchmarked
same-process A/B and all measured neutral-to-worse (the intro is
aggregate-DMA-bound under the power ramp; reordering just moves the
famine), so the previous schedule is kept as-is.
"""

import numpy as np
from contextlib import ExitStack

S, T, B, H = 2048, 2048, 16, 512
NCORES = 8
BLOC = B // NCORES  # batches per core
P128 = 128
NS = S // P128  # 16 s-tiles
NT = T // P128  # 16 t-tiles
NH = H // P128  # 4 h-chunks of the contraction
SC = 512  # s-chunk width (scores tile columns)
NSC = S // SC  # 4 s-chunks per t-tile

_cached_nc = None


def _build():
    import concourse.bacc as bacc
    import concourse.tile as tile
    from concourse import mybir
    from concourse.masks import make_identity

    f32 = mybir.dt.float32
    f16 = mybir.dt.float16

    nc = bacc.Bacc(None, target_bir_lowering=False)
    d_oe = nc.dram_tensor("out_e", [S, BLOC, 2 * H], f32, kind="ExternalInput")
    d_od = nc.dram_tensor("out_d", [T, BLOC, H], f32, kind="ExternalInput")
    d_out = nc.dram_tensor("out", [T, BLOC, H], f32, kind="ExternalOutput")

    with ExitStack() as ctx:
        tc = ctx.enter_context(tile.TileContext(nc))
        singles = ctx.enter_context(tc.tile_pool(name="singles", bufs=1))
        loads = ctx.enter_context(tc.tile_pool(name="loads", bufs=8))
        persist = ctx.enter_context(tc.tile_pool(name="persist", bufs=2))
        work = ctx.enter_context(tc.tile_pool(name="work", bufs=4))
        ptile = ctx.enter_context(tc.tile_pool(name="ptile", bufs=2))
        outs = ctx.enter_context(tc.tile_pool(name="outs", bufs=3))
        small = ctx.enter_context(tc.tile_pool(name="small", bufs=3))
        # PSUM: 8 banks = ps_s0..3 (4) + tr (2) + ps_c (2)
        ps_s_pool = ctx.enter_context(tc.tile_pool(name="ps_s_pool", bufs=1, space="PSUM"))
        ps_tr = ctx.enter_context(tc.tile_pool(name="ps_tr", bufs=2, space="PSUM"))
        ps_cp = ctx.enter_context(tc.tile_pool(name="ps_cp", bufs=2, space="PSUM"))

        id16 = singles.tile([P128, P128], f16)
        make_identity(nc, id16)

        # per-batch persistent tile handles (persist pool tags rotate
        # bufs=2 slots, so consecutive batches double-buffer)
        st = [dict(oeT=[None] * NSC, odT=[None] * NT, nat=[None] * NS,
                   odf=[None] * NT) for _ in range(BLOC)]

        raws = {}

        def oe_dma(b, k, eng=None):
            raw = loads.tile([P128, 2 * H], f32, tag="raw", name="raw", bufs=10)
            (eng or nc.sync).dma_start(
                out=raw, in_=d_oe[k * P128:(k + 1) * P128, b, :]
            )
            raws[(b, k)] = raw

        def oe_add(b, k):
            raw = raws.pop((b, k))
            nat = persist.tile([P128, H], f16, tag=f"oenat{k}", name=f"oenat{k}")
            nc.gpsimd.tensor_add(nat, raw[:, 0:H], raw[:, H:2 * H])
            st[b]["nat"][k] = nat

        def oe_load(b, k):
            if (b, k) not in raws:
                oe_dma(b, k)
            oe_add(b, k)

        def oe_tr(b, k):
            nat = st[b]["nat"][k]
            ci, j = k // 4, k % 4
            if st[b]["oeT"][ci] is None or j == 0:
                st[b]["oeT"][ci] = persist.tile(
                    [P128, NH, SC], f16, tag=f"oeT{ci}", name=f"oeT{ci}"
                )
            trp = ps_tr.tile([P128, H], f16, tag="tr", name="tr_oe")
            for hc in range(NH):
                nc.tensor.transpose(
                    trp[:, hc * P128:(hc + 1) * P128],
                    nat[:, hc * P128:(hc + 1) * P128],
                    id16,
                )
            dst = st[b]["oeT"][ci][:, :, j * P128:(j + 1) * P128]
            src = trp.rearrange("p (h s) -> p h s", h=NH)
            nc.scalar.copy(dst, src)

        def od_load(b, tt):
            odr = loads.tile([P128, H], f32, tag="odr", name="odr")
            nc.sync.dma_start(out=odr, in_=d_od[tt * P128:(tt + 1) * P128, b, :])
            odf = work.tile([P128, H], f16, tag="odf", name="odf")
            nc.gpsimd.tensor_copy(odf, odr)
            st[b]["odf"][tt] = odf

        def od_tr(b, tt):
            odf = st[b]["odf"][tt]
            trp = ps_tr.tile([P128, H], f16, tag="tr", name="tr_od")
            for hc in range(NH):
                nc.tensor.transpose(
                    trp[:, hc * P128:(hc + 1) * P128],
                    odf[:, hc * P128:(hc + 1) * P128],
                    id16,
                )
            odT = persist.tile([P128, NH, P128], f16, tag=f"odT{tt}", name=f"odT{tt}")
            nc.scalar.copy(odT, trp.rearrange("p (h t) -> p h t", h=NH))
            st[b]["odT"][tt] = odT

        def ptr_stage(prev):
            """PE transposes of P(g-1) [t,s]->[s,t] + DVE PSUM->SBUF copies."""
            _, _, pts, _ = prev
            pTs = []
            for half in range(2):
                ptr = ps_tr.tile([P128, 2, SC], f16, tag="tr", name="ptr")
                for sub in range(2):
                    ci = 2 * half + sub
                    for j in range(SC // P128):
                        nc.tensor.transpose(
                            ptr[:, sub, j * P128:(j + 1) * P128],
                            pts[ci][:, j * P128:(j + 1) * P128],
                            id16,
                        )
                pT = ptile.tile([P128, 2 * SC], f16, tag=f"pT{half}", name=f"pT{half}")
                nc.vector.tensor_copy(pT, ptr.rearrange("p a b -> p (a b)"))
                pTs.append(pT)
            return pTs

        def mm1_chunk(b, tt, ci, mx, pss_list):
            pss = ps_s_pool.tile([P128, SC], f32, tag=f"ps_s{ci}", name=f"ps_s{ci}")
            odT = st[b]["odT"][tt]
            oeT = st[b]["oeT"][ci]
            for hc in range(NH):
                nc.tensor.matmul(
                    pss,
                    odT[:, hc, :],
                    oeT[:, hc, :],
                    start=(hc == 0),
                    stop=(hc == NH - 1),
                )
            nc.vector.reduce_max(mx[:, ci:ci + 1], pss, axis=mybir.AxisListType.X)
            pss_list.append(pss)

        def mm1(b, tt):
            mx = small.tile([P128, NSC], f32, tag="mx", name="mx")
            pss_list = []
            for ci in range(NSC):
                mm1_chunk(b, tt, ci, mx, pss_list)
            return mx, pss_list

        def softmax_exp(b, tt, mx, pss_list, order=None):
            m = small.tile([P128, 1], f32, tag="m", name="m")
            nc.vector.reduce_max(m, mx, axis=mybir.AxisListType.X)
            neg_m = small.tile([P128, 1], f32, tag="neg_m", name="neg_m")
            nc.vector.tensor_scalar_mul(neg_m, m, -1.0)
            lacc = small.tile([P128, NSC], f32, tag="lacc", name="lacc")
            pts = [None] * NSC
            for ci in (order if order is not None else range(NSC)):
                pt = ptile.tile([P128, SC], f16, tag=f"pt{ci}", name=f"pt{ci}")
                nc.scalar.activation(
                    pt, pss_list[ci],
                    mybir.ActivationFunctionType.Exp,
                    bias=neg_m, scale=1.0,
                    accum_out=lacc[:, ci:ci + 1],
                )
                pts[ci] = pt
            l = small.tile([P128, 1], f32, tag="l", name="l")
            nc.vector.reduce_sum(l, lacc, axis=mybir.AxisListType.X)
            linv = small.tile([P128, 1], f32, tag="linv", name="linv")
            nc.vector.reciprocal(linv, l)
            return b, tt, pts, linv

        def mm2_mms(prev, pTs):
            pb, _, _, _ = prev
            ps_c = ps_cp.tile([P128, H], f32, tag="ps_c", name="ps_c")
            nat = st[pb]["nat"]
            for k in range(NS):
                nc.tensor.matmul(
                    ps_c,
                    pTs[k // 8][:, (k % 8) * P128:(k % 8 + 1) * P128],
                    nat[k],
                    start=(k == 0), stop=(k == NS - 1),
                )
            return ps_c

        def mm2_out(prev, ps_c):
            pb, ptt, _, plinv = prev
            ot = outs.tile([P128, H], f32, tag="ot", name="ot")
            nc.scalar.activation(
                ot, ps_c, mybir.ActivationFunctionType.Identity,
                bias=0.0, scale=plinv,
            )
            nc.sync.dma_start(
                out=d_out[ptt * P128:(ptt + 1) * P128, pb, :], in_=ot
            )

        def mm2(prev, pTs):
            mm2_out(prev, mm2_mms(prev, pTs))

        # ---- intro: batch 0 tile 0, chunk-interleaved with the oe loads
        # (DMA-bound; the PE stalls here are unavoidable) ----
        # first two oe DMAs go out on the scalar hwdge queue so their
        # descriptor generation doesn't serialize behind od0 on sync
        oe_dma(0, 0, eng=nc.scalar)
        oe_dma(0, 1, eng=nc.scalar)
        od_load(0, 0)
        mx0 = small.tile([P128, NSC], f32, tag="mx", name="mx")
        mx1 = small.tile([P128, NSC], f32, tag="mx", name="mx1")
        pss0 = []
        ps1_hi = {}
        HC2 = SC // 2
        for ci in range(NSC):
            if ci < 2:
                # chunks 0/1 land in the fully DMA-starved region: run
                # N=256 sub-matmuls per 2-tile arrival so the PE starts
                # (and HAM warms) as early as possible
                pss = ps_s_pool.tile(
                    [P128, SC], f32, tag=f"ps_s{ci}", name=f"ps_s{ci}"
                )
                for j in range(2):
                    for k in range(4 * ci + 2 * j, 4 * ci + 2 * j + 2):
                        oe_load(0, k)
                        oe_tr(0, k)
                    if ci == 0 and j == 0:
                        od_tr(0, 0)
                        od_load(0, 1)
                    for hc in range(NH):
                        nc.tensor.matmul(
                            pss[:, j * HC2:(j + 1) * HC2],
                            st[0]["odT"][0][:, hc, :],
                            st[0]["oeT"][ci][:, hc, j * HC2:(j + 1) * HC2],
                            start=(hc == 0),
                            stop=(hc == NH - 1),
                            skip_group_check=True,
                        )
                nc.vector.reduce_max(
                    mx0[:, ci:ci + 1], pss, axis=mybir.AxisListType.X
                )
                pss0.append(pss)
                if ci == 1:
                    od_tr(0, 1)
                continue
            for k in range(4 * ci, 4 * ci + 4):
                oe_load(0, k)
                oe_tr(0, k)
            mm1_chunk(0, 0, ci, mx0, pss0)
            if ci >= 2:
                # tile 1's chunks 2/3 fill the DMA-paced intro using the
                # ps_c banks (idle until the first mm2 in the bridge)
                pss = ps_cp.tile([P128, SC], f32, tag="ps_c", name="ps_c_t1")
                for hc in range(NH):
                    nc.tensor.matmul(
                        pss,
                        st[0]["odT"][1][:, hc, :],
                        st[0]["oeT"][ci][:, hc, :],
                        start=(hc == 0),
                        stop=(hc == NH - 1),
                    )
                nc.vector.reduce_max(
                    mx1[:, ci:ci + 1], pss, axis=mybir.AxisListType.X
                )
                ps1_hi[ci] = pss
        od_load(0, 2)
        prev0 = softmax_exp(0, 0, mx0, pss0)
        od_tr(0, 2)  # fills the PE wait on exp(0,c0) freeing ps_s0
        # tile 1 chunks 0/1 into the ps_s banks just freed by exp(0,c0/c1)
        pss1 = []
        mm1_chunk(0, 1, 0, mx1, pss1)
        mm1_chunk(0, 1, 1, mx1, pss1)
        pss1 += [ps1_hi[2], ps1_hi[3]]
        # exp(1) reads c2/c3 (ps_c banks) first so the bridge mm2's ps_c
        # slot reuse isn't stalled
        prev1 = softmax_exp(0, 1, mx1, pss1, order=(2, 3, 0, 1))
        # bridge: P^T(t0) + mm2(t0), plus od prep for the g=2 loop start
        pTs = ptr_stage(prev0)
        mm2(prev0, pTs)
        od_load(0, 3)
        prev = prev1

        # ---- steady state ----
        # next-batch oe pipeline: dma at iteration END of tt=4..11, DVE
        # adds at tt=5..12 (just after the P^T copies in the DVE FIFO, deps
        # already on-chip), PE transposes at tt=7..14.
        for g in range(2, BLOC * NT):
            b, tt = divmod(g, NT)
            # PE front: P^T(g-1) then MM1(g); the prep transposes go
            # after MM1 so their ps_tr slot reuse never stalls behind the
            # P^T PSUM->SBUF copies (the slots have drained by then)
            pTs = ptr_stage(prev)
            if b + 1 < BLOC and 5 <= tt < 13:
                oe_add(b + 1, 2 * (tt - 5))
                oe_add(b + 1, 2 * (tt - 5) + 1)
            mx, pss_list = mm1(b, tt)
            cur = softmax_exp(b, tt, mx, pss_list)  # ACT: exps before outscale
            if tt + 1 < NT:
                od_tr(b, tt + 1)
            elif b + 1 < BLOC:
                od_tr(b + 1, 0)
            if b + 1 < BLOC and 7 <= tt < 15:
                oe_tr(b + 1, 2 * (tt - 7))
                oe_tr(b + 1, 2 * (tt - 7) + 1)
            mm2(prev, pTs)
            prev = cur
            # iteration tail: DMA issues for future iterations
            if tt + 2 < NT:
                od_load(b, tt + 2)
            elif b + 1 < BLOC and tt + 2 - NT < 2:
                od_load(b + 1, tt + 2 - NT)
            if b + 1 < BLOC and 3 <= tt < 11:
                oe_dma(b + 1, 2 * (tt - 3))
                oe_dma(b + 1, 2 * (tt - 3) + 1)
        pTs = ptr_stage(prev)
        mm2(prev, pTs)

    nc.finalize()
    return nc


def _ensure_devices():
    """Make sure the 8 NeuronCores are visible to jax.devices().

    The calling harness may have pinned jax to cpu (JAX_PLATFORMS=cpu is a
    common pin for running the jax reference); the Bass SPMD launcher uses
    jax.devices(), so re-point jax at the neuron platform if needed.
    """
    import os
    import jax

    try:
        devs = jax.devices()
    except Exception:
        devs = []
    if sum(1 for d in devs if d.platform != "cpu") >= NCORES:
        return
    for plats in ("axon,cpu", None):
        try:
            if plats is None:
                os.environ.pop("JAX_PLATFORMS", None)
            else:
                os.environ["JAX_PLATFORMS"] = plats
            jax.config.update("jax_platforms", plats)
            from jax.extend.backend import clear_backends

            clear_backends()
            devs = jax.devices()
            if sum(1 for d in devs if d.platform != "cpu") >= NCORES:
                return
        except Exception:
            continue


def kernel(in_e=None, out_e=None, out_d=None, **kwargs):
    global _cached_nc
    from concourse.bass_utils import run_bass_kernel_spmd

    _ensure_devices()

    out_e = np.asarray(out_e, dtype=np.float32)
    out_d = np.asarray(out_d, dtype=np.float32)
    if _cached_nc is None:
        _cached_nc = _build()
    in_maps = []
    for c in range(NCORES):
        bsl = slice(c * BLOC, (c + 1) * BLOC)
        in_maps.append({
            "out_e": np.ascontiguousarray(out_e[:, bsl, :]),
            "out_d": np.ascontiguousarray(out_d[:, bsl, :]),
        })
    res = run_bass_kernel_spmd(_cached_nc, in_maps, list(range(NCORES)))
    return np.concatenate([res.results[c]["out"] for c in range(NCORES)], axis=1)
